# revision 2
# baseline (speedup 1.0000x reference)
"""Trainium2 Bass kernel for complex-valued multi-head attention with key masking.

Problem (hardcoded shapes): B=4, Nq=Nk=1024, R=256, NH=8, DK=DV=64.
  Q,K,V complex [B,N,R] (given as _real/_imag f32 pairs), complex weights
  WQ/WK/WV [512,256], WO [256,512], boolean key mask [B,Nk].
  out = complex MHA(Q,K,V) with softmax over |scores| restricted to valid keys.

Sharding: 8 cores = (batch b in 0..3) x (head-group hg in 0..1, 4 heads each).
Each core computes its batch's attention for its 4 heads plus the partial
output projection; the host sums the two head-group partials per batch.

Device-side layout trick: everything is computed "transposed" (channels on
partitions, sequence on the free dim).  Complex arithmetic is folded into the
matmuls by stacking real/imag parts along the 128-partition contraction dim:
  Qa = [Qp_r^T; Qp_i^T], Qb = [Qp_i^T; -Qp_r^T], Ka = [Kp_r^T; Kp_i^T]
  => Sr = Qa^T.Ka,  Si = Qb^T.Ka  (full 128-lane contraction per head)
Softmax runs in [q, k] layout: |s|^2 via a fused custom DVE op, sqrt+exp on
the scalar engine (exp's accum_out gives the denominator for free), masked
keys removed by host-side compaction (padded keys contribute exp(0)=1 to the
denominator, subtracted via a host-provided count).  The normalized weights
are bounced through DRAM with a DMA-transpose load to get E^T for the
attention matmul, which accumulates attn^T = sum_k Vk . E^T in PSUM.
"""

import numpy as np
import ml_dtypes

B, NQ, NK, R = 4, 1024, 1024, 256
NH, DK, DV = 8, 64, 64
NCORES = 8
NHL = 4          # heads per core
F32MIN_PAD = 640  # minimum padded key count (keys padded to a multiple of 128)

_BF16 = ml_dtypes.bfloat16

# ----------------------------------------------------------------------------
# custom DVE ops (registered at import into concourse's op table)
# ----------------------------------------------------------------------------
_OPS = {}


def _register_custom_ops():
    if _OPS:
        return
    import concourse.dve_ops as dom
    from concourse.dve_ops import DveOp
    from concourse.dve_spec import Spec, Src0, Src1, C0, sq, lower, _has_src1
    from concourse.dve_uop import DveOpSpec

    def make(name, spec):
        if name in dom._SUB_OPCODE_FOR_NAME:
            _OPS[name] = next(o for o in dom.OPS if o.name == name)
            return
        row = dom._CUSTOM_DVE_ROW_BASE + len(dom.OPS)
        assert row < 0x20, "custom DVE row overflow"
        shas = {}
        for ver in ("v3", "v4"):
            tmp = DveOpSpec(name=name, opcode=row, uops=lower(spec, ver=ver),
                            rd1_en=_has_src1(spec))
            shas[ver] = tmp.sha(ver)
        op = DveOp(name, spec, subdim=False, uops_sha=shas)
        dom.OPS.append(op)
        dom._SUB_OPCODE_FOR_NAME[name] = row
        dom.CUSTOM_DVE_SPECS[name] = spec
        _OPS[name] = op

    # t = (in0*s0)^2          (drains+squares one score tile from PSUM)
    make("CMHA_SQSC", Spec(
        body=sq(Src0 * C0),
        reference=lambda in0, in1, s0, s1, imm2: (in0.astype(np.float32) * s0) ** 2,
    ))
    # v = (in0*s0)^2 + in1    (second square, accumulate |s|^2)
    make("CMHA_SQADD", Spec(
        body=sq(Src0 * C0) + Src1,
        reference=lambda in0, in1, s0, s1, imm2: (in0.astype(np.float32) * s0) ** 2
        + in1.astype(np.float32),
    ))


# ----------------------------------------------------------------------------
# device program
# ----------------------------------------------------------------------------
_BUILD_CACHE = {}


def _build(nkp):
    """Build + compile the SPMD device program for padded key count nkp."""
    if nkp in _BUILD_CACHE:
        return _BUILD_CACHE[nkp]
    _register_custom_ops()
    import concourse.bass as bass
    import concourse.bacc as bacc
    import concourse.mybir as mybir
    import concourse.tile as tile
    from contextlib import ExitStack

    F32 = mybir.dt.float32
    F32R = mybir.dt.float32r
    BF16 = mybir.dt.bfloat16
    AF = mybir.ActivationFunctionType
    KB = nkp // 128
    assert nkp % 128 == 0

    nc = bacc.Bacc("TRN2", target_bir_lowering=False, debug=False,
                   num_devices=NCORES)

    qt = nc.dram_tensor("qt", [512, NQ], F32, kind="ExternalInput").ap()
    kt = nc.dram_tensor("kt", [512, nkp], F32, kind="ExternalInput").ap()
    vt = nc.dram_tensor("vt", [512, nkp], F32, kind="ExternalInput").ap()
    wq = nc.dram_tensor("wq", [NHL, 512, 256], F32, kind="ExternalInput").ap()
    wk = nc.dram_tensor("wk", [NHL, 512, 128], F32, kind="ExternalInput").ap()
    wv = nc.dram_tensor("wv", [512, 512], F32, kind="ExternalInput").ap()
    wo = nc.dram_tensor("wo", [NHL, 128, 512], BF16, kind="ExternalInput").ap()
    npn = nc.dram_tensor("npn", [128, 1], F32, kind="ExternalInput").ap()
    outr = nc.dram_tensor("outr", [256, NQ], F32, kind="ExternalOutput").ap()
    outi = nc.dram_tensor("outi", [256, NQ], F32, kind="ExternalOutput").ap()

    sqsc = _OPS["CMHA_SQSC"]
    sqadd = _OPS["CMHA_SQADD"]

    with tile.TileContext(nc) as tc, ExitStack() as ctx:
        const = ctx.enter_context(tc.tile_pool(name="const", bufs=1))
        psum = ctx.enter_context(tc.tile_pool(name="psum", bufs=4, space="PSUM"))
        stk = ctx.enter_context(tc.tile_pool(name="stk", bufs=2))
        sm = ctx.enter_context(tc.tile_pool(name="sm", bufs=3))
        etp = ctx.enter_context(tc.tile_pool(name="etp", bufs=6))
        drp = ctx.enter_context(tc.tile_pool(name="drp", bufs=2, space="DRAM"))
        outp = ctx.enter_context(tc.tile_pool(name="outp", bufs=4))

        # ---- input loads -------------------------------------------------
        def load(shape, dtype, src, tag):
            t = const.tile(shape, dtype, tag=tag, name=tag)
            if dtype == F32R:
                nc.sync.dma_start(t[:], src.bitcast(F32R))
            else:
                nc.sync.dma_start(t[:], src)
            return t

        qt_sb = [load([128, NQ], F32R, qt[c * 128:(c + 1) * 128, :], f"qt{c}")
                 for c in range(4)]
        kt_sb = [load([128, nkp], F32R, kt[c * 128:(c + 1) * 128, :], f"kt{c}")
                 for c in range(4)]
        vt_sb = [load([128, nkp], F32R, vt[c * 128:(c + 1) * 128, :], f"vt{c}")
                 for c in range(4)]
        wq_sb = [[load([128, 256], F32R, wq[h, c * 128:(c + 1) * 128, :], f"wq{h}_{c}")
                  for c in range(4)] for h in range(NHL)]
        wk_sb = [[load([128, 128], F32R, wk[h, c * 128:(c + 1) * 128, :], f"wk{h}_{c}")
                  for c in range(4)] for h in range(NHL)]
        wv_sb = [load([128, 512], F32R, wv[c * 128:(c + 1) * 128, :], f"wv{c}")
                 for c in range(4)]
        wo_sb = [load([128, 512], BF16, wo[h], f"wo{h}") for h in range(NHL)]
        npn_sb = load([128, 1], F32, npn[:], "npn")

        VK = const.tile([128, 512 * KB], BF16, tag="vk", name="VK")
        ATT = const.tile([128, NQ * NHL], BF16, tag="att", name="ATT")

        def mmr(out_ap, lhsT, rhs, start, stop):
            nc.tensor.matmul(out_ap, lhsT.bitcast(F32R), rhs.bitcast(F32R),
                             start=start, stop=stop)

        # ---- V projection: VK[k, (h,dstack)] for all 4 heads -------------
        for kb in range(KB):
            ps = psum.tile([128, 1024], F32, tag="ps", name="vk_ps")
            for c in range(4):
                mmr(ps[:, 0:512], vt_sb[c][:, kb * 128:(kb + 1) * 128],
                    wv_sb[c][:], c == 0, c == 3)
            nc.scalar.copy(VK[:, kb * 512:(kb + 1) * 512], ps[:, 0:512])

        # ---- per-head pipeline -------------------------------------------
        for h in range(NHL):
            # projections (f32r, contraction 512 in 4 chunks)
            Qa_ps = psum.tile([128, 1024], F32, tag="ps", name="qa_ps")
            for qc in range(2):
                for c in range(4):
                    mmr(Qa_ps[:, qc * 512:(qc + 1) * 512],
                        wq_sb[h][c][:, 0:128],
                        qt_sb[c][:, qc * 512:(qc + 1) * 512], c == 0, c == 3)
            Qa = stk.tile([128, NQ], BF16, tag="qa", name="Qa")
            nc.scalar.copy(Qa[:], Qa_ps[:])

            Qb_ps = psum.tile([128, 1024], F32, tag="ps", name="qb_ps")
            for qc in range(2):
                for c in range(4):
                    mmr(Qb_ps[:, qc * 512:(qc + 1) * 512],
                        wq_sb[h][c][:, 128:256],
                        qt_sb[c][:, qc * 512:(qc + 1) * 512], c == 0, c == 3)
            Qb = stk.tile([128, NQ], BF16, tag="qb", name="Qb")
            nc.vector.tensor_copy(Qb[:], Qb_ps[:])

            Ka_ps = psum.tile([128, 1024], F32, tag="ps", name="ka_ps")
            for kc in range(KB // 4 + (1 if KB % 4 else 0)):
                pass
            # k free dim nkp: chunks of <=512
            kchunks = [(o, min(512, nkp - o)) for o in range(0, nkp, 512)]
            for (o, w_) in kchunks:
                for c in range(4):
                    mmr(Ka_ps[:, o:o + w_], wk_sb[h][c][:],
                        kt_sb[c][:, o:o + w_], c == 0, c == 3)
            Ka = stk.tile([128, nkp], BF16, tag="ka", name="Ka")
            nc.scalar.copy(Ka[:], Ka_ps[:, 0:nkp])

            # scores + softmax per 128-query block
            esc = drp.tile([NQ, nkp], BF16, tag="esc", name="esc")
            for qb in range(8):
                qa_sl = Qa[:, qb * 128:(qb + 1) * 128]
                qb_sl = Qb[:, qb * 128:(qb + 1) * 128]
                Sr = psum.tile([128, 1024], F32, tag="ps", name="Sr")
                Si = psum.tile([128, 1024], F32, tag="ps", name="Si")
                for (o, w_) in kchunks:
                    nc.tensor.matmul(Sr[:, o:o + w_], qa_sl, Ka[:, o:o + w_],
                                     start=True, stop=True)
                for (o, w_) in kchunks:
                    nc.tensor.matmul(Si[:, o:o + w_], qb_sl, Ka[:, o:o + w_],
                                     start=True, stop=True)
                t = sm.tile([128, nkp], BF16, tag="t", name="t")
                nc.vector._custom_dve(sqsc, out=t[:], in0=Sr[:, 0:nkp], s0=0.125)
                v = sm.tile([128, nkp], BF16, tag="v", name="v")
                nc.vector._custom_dve(sqadd, out=v[:], in0=Si[:, 0:nkp],
                                      in1=t[:], s0=0.125)
                w = sm.tile([128, nkp], BF16, tag="w", name="w")
                nc.scalar.activation(w[:], v[:], AF.Sqrt)
                e = sm.tile([128, nkp], BF16, tag="e", name="e")
                den = sm.tile([128, 1], F32, tag="den", name="den")
                nc.scalar.activation(e[:], w[:], AF.Exp, accum_out=den[:])
                dadj = sm.tile([128, 1], F32, tag="dadj", name="dadj")
                nc.vector.tensor_scalar_add(dadj[:], den[:], npn_sb[:])
                rec = sm.tile([128, 1], F32, tag="rec", name="rec")
                nc.vector.reciprocal(rec[:], dadj[:])
                en = sm.tile([128, nkp], BF16, tag="en", name="en")
                nc.vector.tensor_scalar_mul(en[:], e[:], rec[:])
                nc.sync.dma_start(esc[qb * 128:(qb + 1) * 128, :], en[:])

            # attention: attnT[dstack, q] accumulated over key blocks
            attn_ps = psum.tile([128, 1024], F32, tag="ps", name="attn_ps")
            for kb in range(KB):
                et = etp.tile([128, NQ], BF16, tag="et", name="et")
                nc.sync.dma_start(et[:], esc[:, kb * 128:(kb + 1) * 128],
                                  transpose=True)
                for qc in range(2):
                    nc.tensor.matmul(
                        attn_ps[:, qc * 512:(qc + 1) * 512],
                        VK[:, kb * 512 + h * 128: kb * 512 + (h + 1) * 128],
                        et[:, qc * 512:(qc + 1) * 512],
                        start=(kb == 0), stop=(kb == KB - 1))
            if h % 2 == 0:
                nc.scalar.copy(ATT[:, h * NQ:(h + 1) * NQ], attn_ps[:])
            else:
                nc.vector.tensor_copy(ATT[:, h * NQ:(h + 1) * NQ], attn_ps[:])

        # ---- output projection (accumulate over heads) -------------------
        for ri in range(2):
            for blk in range(2):
                ops_ = psum.tile([128, 1024], F32, tag="ps", name="wo_ps")
                for h in range(NHL):
                    lh = wo_sb[h][:, ri * 256 + blk * 128: ri * 256 + (blk + 1) * 128]
                    for qc in range(2):
                        nc.tensor.matmul(ops_[:, qc * 512:(qc + 1) * 512], lh,
                                         ATT[:, h * NQ + qc * 512: h * NQ + (qc + 1) * 512],
                                         start=(h == 0), stop=(h == NHL - 1))
                osb = outp.tile([128, 1024], F32, tag="osb", name="osb")
                if (ri + blk) % 2 == 0:
                    nc.scalar.copy(osb[:], ops_[:])
                else:
                    nc.vector.tensor_copy(osb[:], ops_[:])
                dst = outr if ri == 0 else outi
                nc.sync.dma_start(dst[blk * 128:(blk + 1) * 128, :], osb[:])

    nc.compile()
    _BUILD_CACHE[nkp] = nc
    return nc


# ----------------------------------------------------------------------------
# host-side prep / gather
# ----------------------------------------------------------------------------
def _prep_inputs(Q_real, Q_imag, K_real, K_imag, V_real, V_imag,
                 WQ_r, WQ_i, WK_r, WK_i, WV_r, WV_i, WO_r, WO_i, mask):
    f32 = np.float32
    mask = np.asarray(mask).astype(bool)
    cnts = mask.sum(1)
    valid = mask.any(1)
    nkp = int(max(F32MIN_PAD, ((int(cnts.max()) + 127) // 128) * 128)) if valid.any() else F32MIN_PAD

    # weight stacks (shared across cores up to head-group slicing)
    A_q = np.concatenate([WQ_r.T, -WQ_i.T], 0).astype(f32)   # [512, 512]
    B_q = np.concatenate([WQ_i.T, WQ_r.T], 0).astype(f32)
    A_k = np.concatenate([WK_r.T, -WK_i.T], 0).astype(f32)
    B_k = np.concatenate([WK_i.T, WK_r.T], 0).astype(f32)
    A_v = np.concatenate([WV_r.T, -WV_i.T], 0).astype(f32)
    B_v = np.concatenate([WV_i.T, WV_r.T], 0).astype(f32)

    in_maps = []
    for core in range(NCORES):
        b, hg = core // 2, core % 2
        idx = np.flatnonzero(mask[b])
        cnt = len(idx)

        def cpad(x):  # [Nk, R] -> gathered+padded [nkp, R]
            out = np.zeros((nkp, R), f32)
            out[:cnt] = x[idx]
            return out

        qtf = np.concatenate([Q_real[b].T, Q_imag[b].T], 0).astype(f32)      # [512, NQ]
        ktf = np.concatenate([cpad(K_real[b]).T, cpad(K_imag[b]).T], 0).astype(f32)
        vtf = np.concatenate([cpad(V_real[b]).T, cpad(V_imag[b]).T], 0).astype(f32)

        wq_l = np.empty((NHL, 512, 256), f32)
        wk_l = np.empty((NHL, 512, 128), f32)
        wv_l = np.empty((512, 512), f32)
        wo_l = np.empty((NHL, 128, 512), _BF16)
        for h in range(NHL):
            g = hg * NHL + h
            gc = slice(g * DK, (g + 1) * DK)
            wq_l[h, :, 0:64] = A_q[:, gc]
            wq_l[h, :, 64:128] = B_q[:, gc]
            wq_l[h, :, 128:192] = B_q[:, gc]
            wq_l[h, :, 192:256] = -A_q[:, gc]
            wk_l[h, :, 0:64] = A_k[:, gc]
            wk_l[h, :, 64:128] = B_k[:, gc]
            wv_l[:, h * 128:h * 128 + 64] = A_v[:, gc]
            wv_l[:, h * 128 + 64:(h + 1) * 128] = B_v[:, gc]
            woa = np.concatenate([WO_r[:, gc].T, -WO_i[:, gc].T], 0)  # [128, 256]
            wob = np.concatenate([WO_i[:, gc].T, WO_r[:, gc].T], 0)
            wo_l[h, :, 0:256] = woa.astype(_BF16)
            wo_l[h, :, 256:512] = wob.astype(_BF16)

        npn = np.full((128, 1), -(nkp - cnt), f32)
        in_maps.append({
            "qt": qtf, "kt": ktf, "vt": vtf,
            "wq": wq_l, "wk": wk_l, "wv": wv_l, "wo": wo_l, "npn": npn,
        })
    return in_maps, nkp, valid


def _gather(results, valid):
    out = np.zeros((B, NQ, R), np.complex64)
    for b in range(B):
        if not valid[b]:
            continue
        r = results[2 * b]["outr"] + results[2 * b + 1]["outr"]   # [256, NQ]
        i = results[2 * b]["outi"] + results[2 * b + 1]["outi"]
        out[b] = (r + 1j * i).T
    return out


def _run(inputs, trace=False, trace_kwargs=None):
    from concourse.bass_utils import run_bass_kernel_spmd
    in_maps, nkp, valid = _prep_inputs(**inputs)
    nc = _build(nkp)
    res = run_bass_kernel_spmd(nc, in_maps, core_ids=list(range(NCORES)),
                               trace=trace, **(trace_kwargs or {}))
    return _gather(res.results, valid), res


def kernel(**inputs) -> np.ndarray:
    out, _ = _run(inputs)
    return out
